# revision 23
# baseline (speedup 1.0000x reference)
"""HRA (Householder Reflection Adaptation) forward kernel for Trainium2.

Math: out = x @ Q with Q = prod_i (I - 2 u_i u_i^T), u_i = normalized columns
of hra_u [4096, 8].  Using the compact WY representation:
    Q = I - U T U^T      (T upper-triangular 8x8, diag=2)
    out = x - (x @ A) @ U^T,   A = U @ T
so the device only does two skinny matmuls per tile plus a subtract.

Sharding: data-parallel over rows. x [4,2048,4096] -> [8192, 4096]; each of
8 cores gets 1024 contiguous rows. A and U^T are tiny and replicated.

Per-core pipeline (128-row tiles, 8 per core):
  DMA-in x_t [128, 4096]
  for each 128-col chunk k: PE-transpose -> PSUM, ACT-copy -> SBUF (x^T)
  proj: 32 accumulating PE matmuls  P^T[8, 128] += A_k^T @ xT_k
  update: 8 PE matmuls  psum[128, 512] = P @ U^T_chunk
  DVE subtract in place, DMA-out
"""

import os
import sys

for _p in ("/opt/trn_rl_repo", "/root/.axon_site", "/root/.axon_site/_ro/trn_rl_repo",
           "/root/.axon_site/_ro/pypackages"):
    if os.path.isdir(_p) and _p not in sys.path:
        sys.path.append(_p)

import numpy as np

import concourse.bass as bass
import concourse.mybir as mybir
import concourse.tile as tile
from concourse import bacc
from concourse.bass_utils import run_bass_kernel_spmd

B, S, D, R = 4, 2048, 4096, 8
N_CORES = 8
ROWS = B * S                      # 8192
ROWS_PER_CORE = ROWS // N_CORES   # 1024
P = 128
N_TILES = ROWS_PER_CORE // P      # 8
D_CHUNKS = D // P                 # 32
UPD_CHUNKS = D // 512             # 8

F32 = mybir.dt.float32

_CACHE = {}


def _householder_wy(hra_u: np.ndarray):
    """Return (A, UT) f32 with out = x - (x @ A) @ UT."""
    u = hra_u.astype(np.float32)
    u = u / np.linalg.norm(u, axis=0, keepdims=True)
    U = u.astype(np.float64)
    T = np.zeros((R, R), np.float64)
    for k in range(R):
        T[k, k] = 2.0
        if k:
            T[:k, k] = -2.0 * (T[:k, :k] @ (U[:, :k].T @ U[:, k]))
    A = (U @ T).astype(np.float32)          # [D, R]
    return A, np.ascontiguousarray(u.T)     # [R, D]


J = 2                             # 128-row tiles per block
BLK = J * P                       # 256 rows per block
N_BLKS = ROWS_PER_CORE // BLK     # 4 blocks per core
F32R = mybir.dt.float32r


def _build_program():
    nc = bacc.Bacc(trn_type="TRN2")
    x = nc.dram_tensor("x", (ROWS_PER_CORE, D), F32, kind="ExternalInput")
    a = nc.dram_tensor("a", (P, D_CHUNKS * R), F32R, kind="ExternalInput")
    ut = nc.dram_tensor("ut", (R, D), F32R, kind="ExternalInput")
    ident = nc.dram_tensor("ident", (P, P), F32, kind="ExternalInput")
    out = nc.dram_tensor("out", (ROWS_PER_CORE, D), F32, kind="ExternalOutput")

    xd = x.rearrange("(b j p) d -> b p j d", p=P, j=J)
    od = out.rearrange("(b j p) d -> b p j d", p=P, j=J)

    with tile.TileContext(nc) as tc:
        with (
            tc.tile_pool(name="const", bufs=1) as const,
            tc.tile_pool(name="xp", bufs=4) as x_pool,
            tc.tile_pool(name="xtp", bufs=4) as xt_pool,
            tc.tile_pool(name="ptp", bufs=3) as pt_pool,
            tc.tile_pool(name="pst", bufs=3, space="PSUM") as pst_pool,
            tc.tile_pool(name="psp", bufs=2, space="PSUM") as psp_pool,
            tc.tile_pool(name="pso", bufs=3, space="PSUM") as pso_pool,
        ):
            # block-0 input first so the first transposes start ASAP
            xbs = []
            xb0 = x_pool.tile([P, J, D], F32, tag="xb")
            xbs.append(xb0)
            h = D // 2
            for j in range(J):
                nc.sync.dma_start(xb0[:, j, :h], xd[0, :, j, :h])
            ident_sb = const.tile([P, P], F32)
            nc.sync.dma_start(ident_sb, ident[:, :])
            for j in range(J):
                nc.sync.dma_start(xb0[:, j, h:], xd[0, :, j, h:])
            a_sb = const.tile([P, D_CHUNKS * R], F32R)
            nc.sync.dma_start(a_sb, a[:, :])
            ut_sb = const.tile([R, D], F32R)
            nc.sync.dma_start(ut_sb, ut[:, :])

            # Prime PE on each constant: hardware allows one sync-wait per
            # LDWEIGHTS, so make PE observe the const DMAs here once instead
            # of stacking const+data waits on the first real matmuls.
            warm_t = pst_pool.tile([P, 2, BLK], F32, tag="ps_t")
            nc.tensor.transpose(warm_t[:, 0, :P], ident_sb, ident_sb)
            warm = pso_pool.tile([P, 512], F32, tag="ps_o")
            nc.tensor.matmul(warm[:R, :P], a_sb[:, :R], a_sb[:, :P],
                             start=True, stop=True)
            nc.tensor.matmul(warm[:, :512], ut_sb[:, :P], ut_sb[:, :512],
                             start=True, stop=True)
            # ~4us of dense matmuls during the initial DMA fill so the PE
            # HAM clock-gate opens before the first real block
            for _ in range(36):
                nc.tensor.matmul(warm[:, :P], ut_sb[:, :P].bitcast(F32R),
                                 ut_sb[:, :P].bitcast(F32R),
                                 start=True, stop=True)

            # prefetch the remaining block inputs
            for b in range(1, N_BLKS):
                xb = x_pool.tile([P, J, D], F32, tag="xb")
                xbs.append(xb)
                for j in range(J):
                    nc.sync.dma_start(xb[:, j], xd[b, :, j])

            def back_units(b, pt, out_piece=UPD_CHUNKS):
                """yield per-(j,c) update+subtract callables; DMA-out every
                `out_piece` chunks (smaller pieces shrink the kernel tail)"""
                xb = xbs[b]

                def unit(j, c):
                    ps_o = pso_pool.tile([P, 512], F32, tag="ps_o")
                    nc.tensor.matmul(
                        ps_o,
                        pt[:, j * P:(j + 1) * P],
                        ut_sb[:, c * 512:(c + 1) * 512],
                        start=True,
                        stop=True,
                    )
                    nc.vector.tensor_sub(
                        xb[:, j, c * 512:(c + 1) * 512],
                        xb[:, j, c * 512:(c + 1) * 512],
                        ps_o,
                    )
                    if (c + 1) % out_piece == 0:
                        lo = (c + 1 - out_piece) * 512
                        hi = (c + 1) * 512
                        nc.scalar.dma_start(od[b, :, j, lo:hi],
                                            xb[:, j, lo:hi])

                for j in range(J):
                    for c in range(UPD_CHUNKS):
                        yield lambda j=j, c=c: unit(j, c)

            def front_units(b):
                """yield per-2-chunk-group callables; pt lands in pts[b]"""
                ps_p = psp_pool.tile([R, BLK], F32, tag="ps_p")

                def group(g):
                    ps_t = pst_pool.tile([P, 2, BLK], F32, tag="ps_t")
                    for i in range(2):
                        k = 2 * g + i
                        for j in range(J):
                            nc.tensor.transpose(
                                ps_t[:, i, j * P:(j + 1) * P],
                                xbs[b][:, j, k * P:(k + 1) * P],
                                ident_sb,
                            )
                    xt_g = xt_pool.tile([P, 2, BLK], F32R, tag="xt_g")
                    nc.scalar.copy(xt_g, ps_t)
                    for i in range(2):
                        k = 2 * g + i
                        nc.tensor.matmul(
                            ps_p,
                            a_sb[:, k * R:(k + 1) * R],
                            xt_g[:, i],
                            start=(k == 0),
                            stop=(k == D_CHUNKS - 1),
                        )

                def finish():
                    pt = pt_pool.tile([R, BLK], F32R, tag="pt")
                    nc.vector.tensor_copy(pt, ps_p)
                    pts[b] = pt

                for g in range(D_CHUNKS // 2):
                    yield lambda g=g: group(g)
                yield lambda: finish()

            def drain(it):
                for f in it:
                    f()

            pts = {}
            drain(front_units(0))
            for b in range(1, N_BLKS):
                fu = list(front_units(b))
                bu = list(back_units(b - 1, pts[b - 1], out_piece=4))
                # front-load: one back unit after each front group until spent
                order = []
                for i, f in enumerate(fu):
                    order.append(f)
                    if i < len(bu):
                        order.append(bu[i])
                drain(order)
            drain(back_units(N_BLKS - 1, pts[N_BLKS - 1], out_piece=2))

    nc.compile()
    return nc


def _get_program():
    if "nc" not in _CACHE:
        _CACHE["nc"] = _build_program()
    return _CACHE["nc"]


def kernel(input, hra_u, **run_kwargs):
    input = np.ascontiguousarray(np.asarray(input, dtype=np.float32))
    hra_u = np.asarray(hra_u, dtype=np.float32)

    A, UT = _householder_wy(hra_u)
    # pack A [D, R] so partition p holds A[c*128+p, :] at free offset c*R
    a_packed = np.ascontiguousarray(
        A.reshape(D_CHUNKS, P, R).transpose(1, 0, 2).reshape(P, D_CHUNKS * R)
    )
    ident = np.eye(P, dtype=np.float32)

    x_flat = input.reshape(ROWS, D)
    in_maps = [
        {
            "x": x_flat[c * ROWS_PER_CORE:(c + 1) * ROWS_PER_CORE],
            "a": a_packed,
            "ut": UT,
            "ident": ident,
        }
        for c in range(N_CORES)
    ]

    nc = _get_program()
    res = run_bass_kernel_spmd(nc, in_maps, core_ids=list(range(N_CORES)),
                               **run_kwargs)
    out = np.concatenate([r["out"] for r in res.results], axis=0)
    if run_kwargs:
        kernel.last_results = res
    return out.reshape(B, S, D)


# revision 25
# speedup vs baseline: 1.0834x; 1.0834x over previous
"""HRA (Householder Reflection Adaptation) forward kernel for Trainium2.

Math: out = x @ Q with Q = prod_i (I - 2 u_i u_i^T), u_i = normalized columns
of hra_u [4096, 8].  Using the compact WY representation:
    Q = I - U T U^T      (T upper-triangular 8x8, diag=2)
    out = x - (x @ A) @ U^T,   A = U @ T
so the device only does two skinny matmuls per tile plus a subtract.

Sharding: data-parallel over rows. x [4,2048,4096] -> [8192, 4096]; each of
8 cores gets 1024 contiguous rows. A and U^T are tiny and replicated.

Per-core pipeline (256-row blocks, 4 per core, software-pipelined):
  all block inputs prefetched up front (SP HWDGE ring)
  front(b): per 2-chunk group: 4 PE transposes -> PSUM strip, ACT copy ->
    SBUF x^T (rounded to f32r), accumulating f32r proj matmul
    P^T[8, 256] += A_k^T @ xT_k  (f32r = single-pass PE fp32, N>=256)
  back(b-1) interleaved into front(b): f32r update matmuls
    psum[128,512] = P @ U^T_chunk, DVE subtract in place, DMA-out on the
    ACT HWDGE ring (reads and writes interleave across SDMA queues)
  a ~4us warm-up matmul burst runs during the initial DMA fill so the PE
  HAM clock-gate opens before the first real block.
"""

import os
import sys

for _p in ("/opt/trn_rl_repo", "/root/.axon_site", "/root/.axon_site/_ro/trn_rl_repo",
           "/root/.axon_site/_ro/pypackages"):
    if os.path.isdir(_p) and _p not in sys.path:
        sys.path.append(_p)

import numpy as np

import concourse.bass as bass
import concourse.mybir as mybir
import concourse.tile as tile
from concourse import bacc
from concourse.bass_utils import run_bass_kernel_spmd

B, S, D, R = 4, 2048, 4096, 8
N_CORES = 8
ROWS = B * S                      # 8192
ROWS_PER_CORE = ROWS // N_CORES   # 1024
P = 128
N_TILES = ROWS_PER_CORE // P      # 8
D_CHUNKS = D // P                 # 32
UPD_CHUNKS = D // 512             # 8

F32 = mybir.dt.float32

_CACHE = {}


def _householder_wy(hra_u: np.ndarray):
    """Return (A, UT) f32 with out = x - (x @ A) @ UT."""
    u = hra_u.astype(np.float32)
    u = u / np.linalg.norm(u, axis=0, keepdims=True)
    U = u.astype(np.float64)
    T = np.zeros((R, R), np.float64)
    for k in range(R):
        T[k, k] = 2.0
        if k:
            T[:k, k] = -2.0 * (T[:k, :k] @ (U[:, :k].T @ U[:, k]))
    A = (U @ T).astype(np.float32)          # [D, R]
    return A, np.ascontiguousarray(u.T)     # [R, D]


J = 2                             # 128-row tiles per block
BLK = J * P                       # 256 rows per block
N_BLKS = ROWS_PER_CORE // BLK     # 4 blocks per core
F32R = mybir.dt.float32r


def _build_program():
    nc = bacc.Bacc(trn_type="TRN2")
    x = nc.dram_tensor("x", (ROWS_PER_CORE, D), F32, kind="ExternalInput")
    a = nc.dram_tensor("a", (P, D_CHUNKS * R), F32R, kind="ExternalInput")
    ut = nc.dram_tensor("ut", (R, D), F32R, kind="ExternalInput")
    ident = nc.dram_tensor("ident", (P, P), F32, kind="ExternalInput")
    out = nc.dram_tensor("out", (ROWS_PER_CORE, D), F32, kind="ExternalOutput")

    xd = x.rearrange("(b j p) d -> b p j d", p=P, j=J)
    od = out.rearrange("(b j p) d -> b p j d", p=P, j=J)

    with tile.TileContext(nc) as tc:
        with (
            tc.tile_pool(name="const", bufs=1) as const,
            tc.tile_pool(name="xp", bufs=4) as x_pool,
            tc.tile_pool(name="xtp", bufs=3) as xt_pool,
            tc.tile_pool(name="ptp", bufs=2) as pt_pool,
            tc.tile_pool(name="pst", bufs=3, space="PSUM") as pst_pool,
            tc.tile_pool(name="psp", bufs=2, space="PSUM") as psp_pool,
            tc.tile_pool(name="pso", bufs=3, space="PSUM") as pso_pool,
        ):
            # block-0 input first so the first transposes start ASAP
            xbs = []
            xb0 = x_pool.tile([P, J, D], F32, tag="xb")
            xbs.append(xb0)
            h = D // 2
            for j in range(J):
                nc.sync.dma_start(xb0[:, j, :h], xd[0, :, j, :h])
            ident_sb = const.tile([P, P], F32)
            nc.sync.dma_start(ident_sb, ident[:, :])
            for j in range(J):
                nc.sync.dma_start(xb0[:, j, h:], xd[0, :, j, h:])
            a_sb = const.tile([P, D_CHUNKS * R], F32R)
            nc.sync.dma_start(a_sb, a[:, :])
            ut_sb = const.tile([R, D], F32R)
            nc.sync.dma_start(ut_sb, ut[:, :])

            # Prime PE on each constant: hardware allows one sync-wait per
            # LDWEIGHTS, so make PE observe the const DMAs here once instead
            # of stacking const+data waits on the first real matmuls.
            warm_t = pst_pool.tile([P, 2, BLK], F32, tag="ps_t")
            nc.tensor.transpose(warm_t[:, 0, :P], ident_sb, ident_sb)
            warm = pso_pool.tile([P, 512], F32, tag="ps_o")
            nc.tensor.matmul(warm[:R, :P], a_sb[:, :R], a_sb[:, :P],
                             start=True, stop=True)
            nc.tensor.matmul(warm[:, :512], ut_sb[:, :P], ut_sb[:, :512],
                             start=True, stop=True)
            # ~4us of dense matmuls during the initial DMA fill so the PE
            # HAM clock-gate opens before the first real block
            for _ in range(36):
                nc.tensor.matmul(warm[:, :P], ut_sb[:, :P].bitcast(F32R),
                                 ut_sb[:, :P].bitcast(F32R),
                                 start=True, stop=True)

            # prefetch the remaining block inputs
            for b in range(1, N_BLKS):
                xb = x_pool.tile([P, J, D], F32, tag="xb")
                xbs.append(xb)
                for j in range(J):
                    nc.sync.dma_start(xb[:, j], xd[b, :, j])

            def back_units(b, pt, out_piece=UPD_CHUNKS):
                """yield per-(j,c) update+subtract callables; DMA-out every
                `out_piece` chunks (smaller pieces shrink the kernel tail)"""
                xb = xbs[b]

                def unit(j, c):
                    ps_o = pso_pool.tile([P, 512], F32, tag="ps_o")
                    nc.tensor.matmul(
                        ps_o,
                        pt[:, j * P:(j + 1) * P],
                        ut_sb[:, c * 512:(c + 1) * 512],
                        start=True,
                        stop=True,
                    )
                    nc.vector.tensor_sub(
                        xb[:, j, c * 512:(c + 1) * 512],
                        xb[:, j, c * 512:(c + 1) * 512],
                        ps_o,
                    )
                    if (c + 1) % out_piece == 0:
                        lo = (c + 1 - out_piece) * 512
                        hi = (c + 1) * 512
                        nc.scalar.dma_start(od[b, :, j, lo:hi],
                                            xb[:, j, lo:hi])

                for j in range(J):
                    for c in range(UPD_CHUNKS):
                        yield lambda j=j, c=c: unit(j, c)

            def front_units(b):
                """yield per-2-chunk-group callables; pt lands in pts[b]"""
                ps_p = psp_pool.tile([R, BLK], F32, tag="ps_p")

                def group(g):
                    ps_t = pst_pool.tile([P, 2, BLK], F32, tag="ps_t")
                    for i in range(2):
                        k = 2 * g + i
                        for j in range(J):
                            nc.tensor.transpose(
                                ps_t[:, i, j * P:(j + 1) * P],
                                xbs[b][:, j, k * P:(k + 1) * P],
                                ident_sb,
                            )
                    xt_g = xt_pool.tile([P, 2, BLK], F32R, tag="xt_g")
                    nc.scalar.copy(xt_g, ps_t)
                    for i in range(2):
                        k = 2 * g + i
                        nc.tensor.matmul(
                            ps_p,
                            a_sb[:, k * R:(k + 1) * R],
                            xt_g[:, i],
                            start=(k == 0),
                            stop=(k == D_CHUNKS - 1),
                        )

                def finish():
                    pt = pt_pool.tile([R, BLK], F32R, tag="pt")
                    nc.vector.tensor_copy(pt, ps_p)
                    pts[b] = pt

                for g in range(D_CHUNKS // 2):
                    yield lambda g=g: group(g)
                yield lambda: finish()

            def drain(it):
                for f in it:
                    f()

            pts = {}
            drain(front_units(0))
            for b in range(1, N_BLKS):
                fu = list(front_units(b))
                bu = list(back_units(b - 1, pts[b - 1], out_piece=2))
                # front-load: one back unit after each front group until spent
                order = []
                for i, f in enumerate(fu):
                    order.append(f)
                    if i < len(bu):
                        order.append(bu[i])
                drain(order)
            drain(back_units(N_BLKS - 1, pts[N_BLKS - 1], out_piece=2))

    nc.compile()
    return nc


def _get_program():
    if "nc" not in _CACHE:
        _CACHE["nc"] = _build_program()
    return _CACHE["nc"]


def kernel(input, hra_u, **run_kwargs):
    input = np.ascontiguousarray(np.asarray(input, dtype=np.float32))
    hra_u = np.asarray(hra_u, dtype=np.float32)

    A, UT = _householder_wy(hra_u)
    # pack A [D, R] so partition p holds A[c*128+p, :] at free offset c*R
    a_packed = np.ascontiguousarray(
        A.reshape(D_CHUNKS, P, R).transpose(1, 0, 2).reshape(P, D_CHUNKS * R)
    )
    ident = np.eye(P, dtype=np.float32)

    x_flat = input.reshape(ROWS, D)
    in_maps = [
        {
            "x": x_flat[c * ROWS_PER_CORE:(c + 1) * ROWS_PER_CORE],
            "a": a_packed,
            "ut": UT,
            "ident": ident,
        }
        for c in range(N_CORES)
    ]

    nc = _get_program()
    res = run_bass_kernel_spmd(nc, in_maps, core_ids=list(range(N_CORES)),
                               **run_kwargs)
    out = np.concatenate([r["out"] for r in res.results], axis=0)
    if run_kwargs:
        kernel.last_results = res
    return out.reshape(B, S, D)
